# revision 17
# baseline (speedup 1.0000x reference)
"""DeepseekV3 decoder layer (MLA + SwiGLU MLP), T=2048 prefill, fp32 I/O.

Sharding v2: tensor-parallel with on-device collectives to minimize
host->device input bytes (~31MB/core vs 187MB replicated).

- Latent projections (q_a, kv_a) run sequence-parallel: core c owns the
  contiguous token strip [c*256, (c+1)*256); results are AllGathered.
- Attention is tensor-parallel over heads: each core computes 2 of 16
  heads for ALL 2048 query tokens; o_proj partial sums are
  ReduceScattered back to token strips (bf16).
- MLP is tensor-parallel over the intermediate dim: each core owns
  1368 (padded 1408) of 10944 columns; hn is AllGathered, down-proj
  partials ReduceScattered.
- LN weights / softmax scale folded into adjacent GEMM weights on host.
"""

import numpy as np
import ml_dtypes

bfloat16 = ml_dtypes.bfloat16

T = 2048
H = 2048
NH = 16
QLR = 1536
KVLR = 512
DN = 128
DR = 64
DV = 128
INTER = 10944
NCORES = 8
TS = T // NCORES           # 256 tokens per strip
NST = TS // 128            # 2 strip tiles
NTT = T // 128             # 16 token tiles
NFC = H // 128             # 16 hidden tiles
NRC = QLR // 128           # 12 q-latent tiles
NKV = KVLR // 128          # 4 kv-latent tiles
HPC = NH // NCORES         # 2 heads per core
IPC = INTER // NCORES      # 1368 intermediate per core
NIT = 11                   # padded local inter tiles (11*128=1408)
IPAD = NIT * 128
EPS = 1e-6
SCALE = (DN + DR) ** -0.5
THETA = 10000.0
QH = DN + DR               # 192

_CACHE = {}


def _build_module():
    import os
    MAXPH = int(os.environ.get("KERNEL_MAXPH", "9"))
    import concourse.bass as bass
    import concourse.tile as tile
    from concourse import bacc, mybir

    f32 = mybir.dt.float32
    bf16 = mybir.dt.bfloat16
    AF = mybir.ActivationFunctionType
    ALU = mybir.AluOpType
    GRP = [list(range(NCORES))]

    nc = bacc.Bacc("TRN2", target_bir_lowering=False, debug=False,
                   enable_asserts=False, num_devices=NCORES)

    def inp(name, shape, dt):
        return nc.dram_tensor(name, list(shape), dt, kind="ExternalInput").ap()

    # per-core inputs
    x_strip = inp("x_strip", [NST, 128, H], f32)
    qb_blk = inp("qb_blk", [NRC, 128, HPC * QH], bf16)
    wuk = inp("wuk", [HPC, 128, NKV, 128], bf16)
    wuv = inp("wuv", [HPC, 128, NKV, DV], bf16)
    ow_blk = inp("ow_blk", [HPC, 128, H], bf16)
    gu_blk = inp("gu_blk", [2, NIT, 128, NFC, 128], bf16)
    dw_blk = inp("dw_blk", [NIT, 128, H], bf16)
    cosk_s = inp("cosk_s", [NST, 128, DR // 2], f32)
    sink_s = inp("sink_s", [NST, 128, DR // 2], f32)
    # replicated inputs
    qa_blk = inp("qa_blk", [NFC, 128, QLR], bf16)
    kva_blk = inp("kva_blk", [NFC, 128, KVLR + DR], bf16)
    cosq = inp("cosq", [NTT, 128, DR // 2], f32)
    sinq = inp("sinq", [NTT, 128, DR // 2], f32)
    trimask = inp("trimask", [128, 128], bf16)
    eye = inp("eye", [128, 128], bf16)
    ones = inp("ones", [128, 1], bf16)

    out_strip = nc.dram_tensor("out_strip", [NST, 128, H], f32,
                               kind="ExternalOutput").ap()

    from contextlib import ExitStack
    with tile.TileContext(nc) as tc, ExitStack() as ctx:
        persist = ctx.enter_context(tc.tile_pool(name="persist", bufs=1))
        dram = ctx.enter_context(
            tc.tile_pool(name="dram", bufs=1, space="DRAM"))

        def pt(shape, dt, tag):
            return persist.tile(list(shape), dt, tag=tag, name=tag)

        eps_sb = pt([128, 1], f32, "eps")
        nc.vector.memset(eps_sb[:], EPS)
        eye_sb = pt([128, 128], bf16, "eye")
        nc.sync.dma_start(out=eye_sb[:], in_=eye[:])
        ones_sb = pt([128, 1], bf16, "ones")
        nc.sync.dma_start(out=ones_sb[:], in_=ones[:])
        tri_sb = pt([128, 128], bf16, "tri")
        nc.sync.dma_start(out=tri_sb[:], in_=trimask[:])
        zero4 = pt([128, 4], bf16, "zero4")
        nc.vector.memset(zero4[:], 0.0)
        x_sb = pt([128, NST, H], f32, "x_sb")
        for st in range(NST):
            nc.sync.dma_start(out=x_sb[:, st, :], in_=x_strip[st])
        h2_sb = pt([128, NST, H], f32, "h2_sb")
        rstd_x = pt([128, NST], f32, "rstd_x")

        # DRAM bounce buffers for collectives
        g_qcT_in = dram.tile([NST, NRC, 128, 128], bf16, name="g_qcT_in")
        g_qcT_out = dram.tile([NCORES, NST, NRC, 128, 128], bf16,
                              name="g_qcT_out", addr_space="Shared")
        g_kvT_in = dram.tile([NST, NKV + 1, 128, 128], bf16, name="g_kvT_in")
        g_kvT_out = dram.tile([NCORES, NST, NKV + 1, 128, 128], bf16,
                              name="g_kvT_out", addr_space="Shared")
        g_ch_in = dram.tile([NST, 128, KVLR], bf16, name="g_ch_in")
        g_ch_out = dram.tile([NCORES, NST, 128, KVLR], bf16,
                             name="g_ch_out", addr_space="Shared")
        g_at_in = dram.tile([NTT, 128, H], bf16, name="g_at_in")
        g_at_out = dram.tile([NST, 128, H], bf16, name="g_at_out")
        g_hnT_in = dram.tile([NST, NFC, 128, 128], bf16, name="g_hnT_in")
        g_hnT_out = dram.tile([NCORES, NST, NFC, 128, 128], bf16,
                              name="g_hnT_out", addr_space="Shared")
        g_mlp_in = dram.tile([NTT, 128, H], bf16, name="g_mlp_in")
        g_mlp_out = dram.tile([NST, 128, H], bf16, name="g_mlp_out")

        # ========== phase L: local latent (own strip) + AllGathers ==========
        with tc.tile_pool(name="pl", bufs=3) as pl, \
             tc.tile_pool(name="pls", bufs=1) as pls, \
             tc.tile_pool(name="pld", bufs=2) as pld:
            # x stats + bf16 copy + transpose
            xbf = pls.tile([128, NST, H], bf16, name="xbf")
            ssq_x = pls.tile([128, NST], f32, name="ssq_x")
            for st in range(NST):
                scr0 = pld.tile([128, H], bf16, tag="scr0", name="scr0")
                nc.scalar.activation(scr0[:], x_sb[:, st, :], AF.Square,
                                     accum_out=ssq_x[:, st:st + 1])
                nc.vector.tensor_copy(xbf[:, st, :], x_sb[:, st, :])
            nc.scalar.activation(rstd_x[:], ssq_x[:], AF.Ln,
                                 bias=eps_sb[:], scale=1.0 / H)
            nc.scalar.activation(rstd_x[:], rstd_x[:], AF.Exp, scale=-0.5)
            xT = pls.tile([128, NFC, TS], bf16, name="xT")

            with tc.tile_pool(name="plkv", bufs=2, space="PSUM") as plkv, \
                 tc.tile_pool(name="pltp", bufs=2, space="PSUM") as pltp:
                for st in range(NST):
                    for fc in range(NFC):
                        tp = pltp.tile([128, 128], bf16, tag="tp", name="tp")
                        nc.tensor.transpose(
                            tp[:], xbf[:, st, fc * 128:(fc + 1) * 128],
                            eye_sb[:])
                        nc.any.tensor_copy(xT[:, fc, st * 128:(st + 1) * 128],
                                           tp[:])
                # kv path
                cosk_sb = pls.tile([128, NST, DR // 2], f32, name="cosk_sb")
                sink_sb = pls.tile([128, NST, DR // 2], f32, name="sink_sb")
                for st in range(NST):
                    nc.sync.dma_start(out=cosk_sb[:, st, :], in_=cosk_s[st])
                    nc.sync.dma_start(out=sink_sb[:, st, :], in_=sink_s[st])
                ssq_kv = pls.tile([128, NST], f32, name="ssq_kv")
                c_raw = pls.tile([128, NST, KVLR + DR], f32, name="c_raw")
                kvw = []
                for fc in range(NFC):
                    w = pl.tile([128, KVLR + DR], bf16, tag="kvw", name="kvw",
                                bufs=NFC)
                    nc.sync.dma_start(out=w[:], in_=kva_blk[fc])
                    kvw.append(w)
                for st in range(NST):
                    ps = plkv.tile([128, KVLR + DR], f32, tag="kvps",
                                   name="kvps")
                    for fc in range(NFC):
                        nc.tensor.matmul(ps[:, 0:512],
                                         xT[:, fc, st * 128:(st + 1) * 128],
                                         kvw[fc][:, 0:512],
                                         start=(fc == 0), stop=(fc == NFC - 1))
                        nc.tensor.matmul(ps[:, 512:576],
                                         xT[:, fc, st * 128:(st + 1) * 128],
                                         kvw[fc][:, 512:576],
                                         start=(fc == 0), stop=(fc == NFC - 1))
                    scr = pld.tile([128, KVLR], bf16, tag="scr", name="scr")
                    nc.scalar.activation(scr[:], ps[:, 0:512], AF.Square,
                                         accum_out=ssq_kv[:, st:st + 1])
                    nc.vector.tensor_copy(c_raw[:, st, :], ps[:])
                t1 = pls.tile([128, NST], f32, name="t1")
                nc.vector.tensor_mul(t1[:], rstd_x[:], rstd_x[:])
                nc.vector.tensor_mul(t1[:], t1[:], ssq_kv[:])
                nc.scalar.activation(t1[:], t1[:], AF.Ln, bias=eps_sb[:],
                                     scale=1.0 / KVLR)
                nc.scalar.activation(t1[:], t1[:], AF.Exp, scale=-0.5)
                nc.vector.tensor_mul(t1[:], rstd_x[:], t1[:])
                c_hat = pls.tile([128, NST, KVLR], bf16, name="c_hat")
                for st in range(NST):
                    nc.vector.tensor_scalar_mul(c_hat[:, st, :],
                                                c_raw[:, st, 0:512],
                                                t1[:, st:st + 1])
                    nc.sync.dma_start(out=g_ch_in[st], in_=c_hat[:, st, :])
                kr = pls.tile([128, NST, DR], f32, name="kr")
                krf = pls.tile([128, NST, DR], bf16, name="krf")
                for st in range(NST):
                    nc.vector.tensor_scalar_mul(kr[:, st, :],
                                                c_raw[:, st, 512:576],
                                                rstd_x[:, st:st + 1])
                x1 = kr[:, :, 0:DR:2]
                x2 = kr[:, :, 1:DR:2]
                ta = pls.tile([128, NST, DR // 2], f32, name="ta")
                tb = pls.tile([128, NST, DR // 2], f32, name="tb")
                nc.vector.tensor_mul(ta[:], x1, cosk_sb[:])
                nc.vector.tensor_mul(tb[:], x2, sink_sb[:])
                nc.vector.tensor_sub(krf[:, :, 0:DR:2], ta[:], tb[:])
                nc.vector.tensor_mul(ta[:], x2, cosk_sb[:])
                nc.vector.tensor_mul(tb[:], x1, sink_sb[:])
                nc.vector.tensor_add(krf[:, :, 1:DR:2], ta[:], tb[:])
                # transposed kv pack: slots 0-3 c_hat^T, slot 4 k_rope^T
                for st in range(NST):
                    for rc in range(NKV):
                        tp = pltp.tile([128, 128], bf16, tag="tp", name="tp")
                        nc.tensor.transpose(
                            tp[:], c_hat[:, st, rc * 128:(rc + 1) * 128],
                            eye_sb[:])
                        stage = pld.tile([128, 128], bf16, tag="stage",
                                         name="stage")
                        nc.any.tensor_copy(stage[:], tp[:])
                        nc.sync.dma_start(out=g_kvT_in[st, rc], in_=stage[:])
                    tp = pltp.tile([128, 128], bf16, tag="tp", name="tp")
                    nc.tensor.transpose(tp[0:64, :], krf[:, st, :], eye_sb[:])
                    stage = pld.tile([128, 128], bf16, tag="stage",
                                     name="stage")
                    nc.any.tensor_copy(stage[0:64, :], tp[0:64, :])
                    nc.sync.dma_start(out=g_kvT_in[st, NKV, 0:64, :],
                                      in_=stage[0:64, :])
                nc.gpsimd.collective_compute(
                    "AllGather", ALU.bypass, replica_groups=GRP,
                    ins=[g_ch_in.opt()], outs=[g_ch_out.opt()])
                nc.gpsimd.collective_compute(
                    "AllGather", ALU.bypass, replica_groups=GRP,
                    ins=[g_kvT_in.opt()], outs=[g_kvT_out.opt()])

            # q path (reuses xT)
            with tc.tile_pool(name="pqps", bufs=2, space="PSUM") as pqps, \
                 tc.tile_pool(name="pqtp", bufs=2, space="PSUM") as pqtp:
                ssq_q = pls.tile([128, NST], f32, name="ssq_q")
                sq = pls.tile([128, NST], f32, name="sq")
                qc_ps = [pqps.tile([128, QLR], f32, tag="qaps", name="qaps")
                         for _ in range(NST)]
                for fc in range(NFC):
                    qaw = pl.tile([128, QLR], bf16, tag="qaw", name="qaw")
                    nc.sync.dma_start(out=qaw[:], in_=qa_blk[fc])
                    for st in range(NST):
                        for nn in range(QLR // 512):
                            nc.tensor.matmul(
                                qc_ps[st][:, nn * 512:(nn + 1) * 512],
                                xT[:, fc, st * 128:(st + 1) * 128],
                                qaw[:, nn * 512:(nn + 1) * 512],
                                start=(fc == 0), stop=(fc == NFC - 1))
                for st in range(NST):
                    scrq = pld.tile([128, QLR], bf16, tag="scrq", name="scrq")
                    nc.scalar.activation(scrq[:], qc_ps[st][:], AF.Square,
                                         accum_out=ssq_q[:, st:st + 1])
                nc.vector.tensor_mul(sq[:], rstd_x[:], rstd_x[:])
                nc.vector.tensor_mul(sq[:], sq[:], ssq_q[:])
                nc.scalar.activation(sq[:], sq[:], AF.Ln, bias=eps_sb[:],
                                     scale=1.0 / QLR)
                nc.scalar.activation(sq[:], sq[:], AF.Exp, scale=-0.5)
                nc.vector.tensor_mul(sq[:], rstd_x[:], sq[:])
                qc = pls.tile([128, NST, QLR], bf16, name="qc")
                for st in range(NST):
                    nc.vector.tensor_scalar_mul(qc[:, st, :], qc_ps[st][:],
                                                sq[:, st:st + 1])
                for st in range(NST):
                    for rc in range(NRC):
                        tp = pqtp.tile([128, 128], bf16, tag="tp", name="tp")
                        nc.tensor.transpose(
                            tp[:], qc[:, st, rc * 128:(rc + 1) * 128],
                            eye_sb[:])
                        stage = pld.tile([128, 128], bf16, tag="stageq",
                                         name="stageq")
                        nc.any.tensor_copy(stage[:], tp[:])
                        nc.sync.dma_start(out=g_qcT_in[st, rc], in_=stage[:])
                nc.gpsimd.collective_compute(
                    "AllGather", ALU.bypass, replica_groups=GRP,
                    ins=[g_qcT_in.opt()], outs=[g_qcT_out.opt()])

        # =================== phase A: attention (2 heads) ===================
        if MAXPH >= 1:
            with tc.tile_pool(name="pas", bufs=1) as pas, \
                 tc.tile_pool(name="pad", bufs=2) as pad:
                # load gathered kv
                kT_lat = pas.tile([128, NKV, T], bf16, name="kT_lat")
                kT_rope = pas.tile([64, T], bf16, name="kT_rope")
                ch_full = pas.tile([128, NTT, KVLR], bf16, name="ch_full")
                for c8 in range(NCORES):
                    for st in range(NST):
                        kt = c8 * NST + st
                        for rc in range(NKV):
                            nc.gpsimd.dma_start(
                                out=kT_lat[:, rc, kt * 128:(kt + 1) * 128],
                                in_=g_kvT_out[c8, st, rc])
                        nc.gpsimd.dma_start(
                            out=kT_rope[:, kt * 128:(kt + 1) * 128],
                            in_=g_kvT_out[c8, st, NKV, 0:64, :])
                        nc.gpsimd.dma_start(out=ch_full[:, kt, :],
                                            in_=g_ch_out[c8, st])
                wuk_sb = pas.tile([128, HPC, NKV, 128], bf16, name="wuk_sb")
                wuv_sb = pas.tile([128, HPC, NKV, DV], bf16, name="wuv_sb")
                for h in range(HPC):
                    nc.sync.dma_start(out=wuk_sb[:, h], in_=wuk[h])
                    nc.sync.dma_start(out=wuv_sb[:, h], in_=wuv[h])

                # q_b + rope + transposes (temporaries scoped)
                qnT = pas.tile([128, HPC, T], bf16, name="qnT")
                qrT = pas.tile([64, HPC, T], bf16, name="qrT")
                with tc.tile_pool(name="paq", bufs=3) as paq, \
                     tc.tile_pool(name="paqs", bufs=1) as paqs, \
                     tc.tile_pool(name="pqb", bufs=2, space="PSUM") as pqb, \
                     tc.tile_pool(name="patp", bufs=2, space="PSUM") as patp:
                    cosq_sb = paqs.tile([128, NTT, DR // 2], f32,
                                        name="cosq_sb")
                    sinq_sb = paqs.tile([128, NTT, DR // 2], f32,
                                        name="sinq_sb")
                    for tt in range(NTT):
                        nc.sync.dma_start(out=cosq_sb[:, tt, :], in_=cosq[tt])
                        nc.sync.dma_start(out=sinq_sb[:, tt, :], in_=sinq[tt])
                    qbw = paqs.tile([128, NRC, HPC * QH], bf16, name="qbw")
                    for rc in range(NRC):
                        nc.sync.dma_start(out=qbw[:, rc, :], in_=qb_blk[rc])
                    for tt in range(NTT):
                        qcT_t = paq.tile([128, NRC, 128], bf16, tag="qcT_t",
                                         name="qcT_t")
                        c8, st = divmod(tt, NST)
                        for rc in range(NRC):
                            nc.sync.dma_start(out=qcT_t[:, rc, :],
                                              in_=g_qcT_out[c8, st, rc])
                        q2 = pqb.tile([128, HPC * QH], f32, tag="q2",
                                      name="q2")
                        for rc in range(NRC):
                            nc.tensor.matmul(q2[:], qcT_t[:, rc, :],
                                             qbw[:, rc, :], start=(rc == 0),
                                             stop=(rc == NRC - 1))
                        qn2 = pad.tile([128, HPC * DN], bf16, tag="qn2",
                                       name="qn2")
                        qrr = pad.tile([128, HPC * DR], f32, tag="qrr",
                                       name="qrr")
                        qr2 = pad.tile([128, HPC * DR], bf16, tag="qr2",
                                       name="qr2")
                        ta = pad.tile([128, DR // 2], f32, tag="taq",
                                      name="taq")
                        tb = pad.tile([128, DR // 2], f32, tag="tbq",
                                      name="tbq")
                        for h in range(HPC):
                            nc.scalar.copy(qn2[:, h * DN:(h + 1) * DN],
                                           q2[:, h * QH:h * QH + DN])
                            nc.scalar.copy(qrr[:, h * DR:(h + 1) * DR],
                                           q2[:, h * QH + DN:(h + 1) * QH])
                        for h in range(HPC):
                            x1 = qrr[:, h * DR + 0:(h + 1) * DR:2]
                            x2 = qrr[:, h * DR + 1:(h + 1) * DR:2]
                            nc.vector.tensor_mul(ta[:], x1, cosq_sb[:, tt, :])
                            nc.vector.tensor_mul(tb[:], x2, sinq_sb[:, tt, :])
                            nc.vector.tensor_sub(
                                qr2[:, h * DR + 0:(h + 1) * DR:2], ta[:],
                                tb[:])
                            nc.vector.tensor_mul(ta[:], x2, cosq_sb[:, tt, :])
                            nc.vector.tensor_mul(tb[:], x1, sinq_sb[:, tt, :])
                            nc.vector.tensor_add(
                                qr2[:, h * DR + 1:(h + 1) * DR:2], ta[:],
                                tb[:])
                        for h in range(HPC):
                            tp = patp.tile([128, 128], bf16, tag="tp",
                                           name="tp")
                            nc.tensor.transpose(
                                tp[:], qn2[:, h * DN:(h + 1) * DN], eye_sb[:])
                            nc.any.tensor_copy(
                                qnT[:, h, tt * 128:(tt + 1) * 128], tp[:])
                            tp = patp.tile([128, 128], bf16, tag="tp",
                                           name="tp")
                            nc.tensor.transpose(
                                tp[0:64, :], qr2[:, h * DR:(h + 1) * DR],
                                eye_sb[:])
                            nc.any.tensor_copy(
                                qrT[:, h, tt * 128:(tt + 1) * 128],
                                tp[0:64, :])

                # per-head: absorb + scores + softmax + o_latent + o_v
                o_vT = pas.tile([128, HPC, T], bf16, name="o_vT")
                qT = pas.tile([128, NKV, T], bf16, tag="qT", name="qT",
                              bufs=1)
                oln = pas.tile([128, NTT, KVLR], bf16, tag="oln", name="oln",
                               bufs=1)
                DBG = bool(os.environ.get("KERNEL_DEBUG"))
                if DBG:
                    dbg_den = pas.tile([128, NTT], f32, name="dbg_den")
                    dbg_eT = pas.tile([128, 2, 512], bf16, name="dbg_eT")
                for h in range(HPC):
                    with tc.tile_pool(name="pab", bufs=2, space="PSUM") as pab:
                        for rc in range(NKV):
                            for ch4 in range(T // 512):
                                lp = pab.tile([128, 512], f32, tag="lp",
                                              name="lp")
                                nc.tensor.matmul(
                                    lp[:], wuk_sb[:, h, rc, :],
                                    qnT[:, h, ch4 * 512:(ch4 + 1) * 512],
                                    start=True, stop=True)
                                nc.scalar.copy(
                                    qT[:, rc, ch4 * 512:(ch4 + 1) * 512],
                                    lp[:])
                    with tc.tile_pool(name="psc", bufs=2,
                                      space="PSUM") as psc, \
                         tc.tile_pool(name="pol", bufs=4,
                                      space="PSUM") as pol, \
                         tc.tile_pool(name="pden", bufs=1,
                                      space="PSUM") as pden:
                        for qc4 in range(T // 512):
                            ol = [pol.tile([128, KVLR], f32, tag="ol",
                                           name="ol") for _ in range(4)]
                            den = pden.tile([128, 4], f32, tag="den",
                                            name="den")
                            # zero the whole den bank once: start=True zeroes
                            # at bank granularity, so interleaved per-column
                            # chains must all accumulate (start=False)
                            nc.tensor.matmul(den[:], eye_sb[:], zero4[:],
                                             start=True, stop=False,
                                             skip_group_check=True)
                            nkt = 4 * qc4 + 4
                            for kt in range(nkt):
                                q0 = max(0, (kt - 4 * qc4) * 128)
                                qw = 512 - q0
                                sp = psc.tile([128, 512], f32, tag="sp",
                                              name="sp")
                                qs = qc4 * 512 + q0
                                for rc in range(NKV):
                                    nc.tensor.matmul(
                                        sp[:, q0:512],
                                        kT_lat[:, rc,
                                               kt * 128:(kt + 1) * 128],
                                        qT[:, rc, qs:qs + qw],
                                        start=(rc == 0), stop=False)
                                nc.tensor.matmul(
                                    sp[:, q0:512],
                                    kT_rope[:, kt * 128:(kt + 1) * 128],
                                    qrT[:, h, qs:qs + qw],
                                    start=False, stop=True)
                                eT = pad.tile([128, 512], bf16, tag="eT",
                                              name="eT", bufs=3)
                                nc.scalar.activation(eT[:, q0:512],
                                                     sp[:, q0:512], AF.Exp)
                                if kt >= 4 * qc4:
                                    nc.vector.tensor_mul(
                                        eT[:, q0:q0 + 128],
                                        eT[:, q0:q0 + 128], tri_sb[:])
                                if DBG and h == 0 and qc4 == 1 and kt < 2:
                                    nc.vector.tensor_copy(dbg_eT[:, kt, :],
                                                          eT[:])
                                for qt4 in range(max(0, kt - 4 * qc4), 4):
                                    qt = 4 * qc4 + qt4
                                    nc.tensor.matmul(
                                        ol[qt4][:],
                                        eT[:, qt4 * 128:(qt4 + 1) * 128],
                                        ch_full[:, kt, :],
                                        start=(kt == 0), stop=(kt == qt))
                                    nc.tensor.matmul(
                                        den[:, qt4:qt4 + 1],
                                        eT[:, qt4 * 128:(qt4 + 1) * 128],
                                        ones_sb[:],
                                        start=False, stop=(kt == qt),
                                        skip_group_check=True)
                            rinv = pad.tile([128, 4], f32, tag="rinv",
                                            name="rinv")
                            nc.vector.reciprocal(rinv[:], den[:])
                            if DBG and h == 0:
                                nc.vector.tensor_copy(
                                    dbg_den[:, qc4 * 4:(qc4 + 1) * 4], den[:])
                            for qt4 in range(4):
                                qt = 4 * qc4 + qt4
                                nc.vector.tensor_scalar_mul(
                                    oln[:, qt, :], ol[qt4][:],
                                    rinv[:, qt4:qt4 + 1])
                    if os.environ.get("KERNEL_DEBUG") and h == 0:
                        d_qT = nc.dram_tensor("d_qT", [128, NKV, T], bf16,
                                              kind="ExternalOutput").ap()
                        nc.sync.dma_start(out=d_qT[:], in_=qT[:])
                        d_oln = nc.dram_tensor("d_oln", [128, NTT, KVLR],
                                               bf16,
                                               kind="ExternalOutput").ap()
                        nc.sync.dma_start(out=d_oln[:], in_=oln[:])
                    # transpose oln + o_v
                    with tc.tile_pool(name="pvt", bufs=2,
                                      space="PSUM") as pvt, \
                         tc.tile_pool(name="pov", bufs=2,
                                      space="PSUM") as pov:
                        for ch4 in range(T // 512):
                            olT = pad.tile([128, NKV, 512], bf16, tag="olT",
                                           name="olT")
                            for qt4 in range(4):
                                qt = 4 * ch4 + qt4
                                for rc in range(NKV):
                                    tp = pvt.tile([128, 128], bf16, tag="tp",
                                                  name="tp")
                                    nc.tensor.transpose(
                                        tp[:],
                                        oln[:, qt, rc * 128:(rc + 1) * 128],
                                        eye_sb[:])
                                    nc.any.tensor_copy(
                                        olT[:, rc,
                                            qt4 * 128:(qt4 + 1) * 128],
                                        tp[:])
                            ovp = pov.tile([128, 512], f32, tag="ovp",
                                           name="ovp")
                            for rc in range(NKV):
                                nc.tensor.matmul(
                                    ovp[:], wuv_sb[:, h, rc, :],
                                    olT[:, rc, :],
                                    start=(rc == 0), stop=(rc == NKV - 1))
                            nc.scalar.copy(
                                o_vT[:, h, ch4 * 512:(ch4 + 1) * 512],
                                ovp[:])

                if os.environ.get("KERNEL_DEBUG"):
                    d_den = nc.dram_tensor("d_den", [128, NTT], f32,
                                           kind="ExternalOutput").ap()
                    nc.sync.dma_start(out=d_den[:], in_=dbg_den[:])
                    d_eT = nc.dram_tensor("d_eT", [128, 2, 512], bf16,
                                          kind="ExternalOutput").ap()
                    nc.sync.dma_start(out=d_eT[:], in_=dbg_eT[:])
                    d_qnT = nc.dram_tensor("d_qnT", [128, HPC, T], bf16,
                                           kind="ExternalOutput").ap()
                    nc.sync.dma_start(out=d_qnT[:], in_=qnT[:])
                    d_qrT = nc.dram_tensor("d_qrT", [64, HPC, T], bf16,
                                           kind="ExternalOutput").ap()
                    nc.sync.dma_start(out=d_qrT[:], in_=qrT[:])
                    d_ovT = nc.dram_tensor("d_ovT", [128, HPC, T], bf16,
                                           kind="ExternalOutput").ap()
                    nc.sync.dma_start(out=d_ovT[:], in_=o_vT[:])

                # o_proj partial + ReduceScatter
                ow_sb = pas.tile([128, HPC, H], bf16, name="ow_sb")
                for h in range(HPC):
                    nc.sync.dma_start(out=ow_sb[:, h, :], in_=ow_blk[h])
                with tc.tile_pool(name="pop", bufs=2, space="PSUM") as pop:
                    for qt in range(NTT):
                        at_ps = pop.tile([128, H], f32, tag="at_ps",
                                         name="at_ps")
                        for h in range(HPC):
                            for nn in range(H // 512):
                                nc.tensor.matmul(
                                    at_ps[:, nn * 512:(nn + 1) * 512],
                                    o_vT[:, h, qt * 128:(qt + 1) * 128],
                                    ow_sb[:, h, nn * 512:(nn + 1) * 512],
                                    start=(h == 0), stop=(h == HPC - 1))
                        at_bf = pad.tile([128, H], bf16, tag="at_bf",
                                         name="at_bf")
                        nc.scalar.copy(at_bf[:], at_ps[:])
                        nc.sync.dma_start(out=g_at_in[qt], in_=at_bf[:])
                nc.gpsimd.collective_compute(
                    "ReduceScatter", ALU.add, replica_groups=GRP,
                    ins=[g_at_in.opt()], outs=[g_at_out.opt()])

        # ============ phase R: residual + post-norm + AG hnT ============
        if MAXPH >= 2:
            with tc.tile_pool(name="pr", bufs=2) as pr, \
                 tc.tile_pool(name="prs", bufs=1) as prs, \
                 tc.tile_pool(name="prtp", bufs=2, space="PSUM") as prtp:
                at_s = prs.tile([128, NST, H], bf16, name="at_s")
                for st in range(NST):
                    nc.sync.dma_start(out=at_s[:, st, :], in_=g_at_out[st])
                ssq2 = prs.tile([128, NST], f32, name="ssq2")
                hn = prs.tile([128, NST, H], bf16, name="hn")
                for st in range(NST):
                    nc.vector.tensor_add(h2_sb[:, st, :], x_sb[:, st, :],
                                         at_s[:, st, :])
                for st in range(NST):
                    scr2 = pr.tile([128, H], bf16, tag="scr2", name="scr2")
                    nc.vector.scalar_tensor_tensor(
                        scr2[:], h2_sb[:, st, :], 1.0, h2_sb[:, st, :],
                        ALU.bypass, ALU.mult, accum_out=ssq2[:, st:st + 1])
                nc.scalar.activation(ssq2[:], ssq2[:], AF.Ln, bias=eps_sb[:],
                                     scale=1.0 / H)
                nc.scalar.activation(ssq2[:], ssq2[:], AF.Exp, scale=-0.5)
                for st in range(NST):
                    nc.vector.tensor_scalar_mul(hn[:, st, :], h2_sb[:, st, :],
                                                ssq2[:, st:st + 1])
                for st in range(NST):
                    for fc in range(NFC):
                        tp = prtp.tile([128, 128], bf16, tag="tp", name="tp")
                        nc.tensor.transpose(
                            tp[:], hn[:, st, fc * 128:(fc + 1) * 128],
                            eye_sb[:])
                        stage = pr.tile([128, 128], bf16, tag="stage3",
                                        name="stage3")
                        nc.any.tensor_copy(stage[:], tp[:])
                        nc.sync.dma_start(out=g_hnT_in[st, fc], in_=stage[:])
                nc.gpsimd.collective_compute(
                    "AllGather", ALU.bypass, replica_groups=GRP,
                    ins=[g_hnT_in.opt()], outs=[g_hnT_out.opt()])

        # =================== phase M: MLP (TP inter) ===================
        if MAXPH >= 3:
            with tc.tile_pool(name="pm", bufs=2) as pm, \
                 tc.tile_pool(name="pmw", bufs=1) as pmw, \
                 tc.tile_pool(name="pmd", bufs=2) as pmd:
                hnT = pmw.tile([128, NFC, T], bf16, name="hnT")
                for c8 in range(NCORES):
                    for st in range(NST):
                        tt = c8 * NST + st
                        for fc in range(NFC):
                            nc.gpsimd.dma_start(
                                out=hnT[:, fc, tt * 128:(tt + 1) * 128],
                                in_=g_hnT_out[c8, st, fc])
                dw_sb = pmw.tile([128, NIT, H], bf16, name="dw_sb")
                for it in range(NIT):
                    nc.sync.dma_start(out=dw_sb[:, it, :], in_=dw_blk[it])
                THALF = T // 2
                actT = pmw.tile([128, NIT, THALF], bf16, name="actT")
                for th in range(2):
                    with tc.tile_pool(name="pmg", bufs=2,
                                      space="PSUM") as pmg, \
                         tc.tile_pool(name="pmu", bufs=2,
                                      space="PSUM") as pmu:
                        for it in range(NIT):
                            gw = pm.tile([128, NFC, 128], bf16, tag="gw",
                                         name="gw")
                            nc.sync.dma_start(out=gw[:], in_=gu_blk[0, it])
                            uw = pm.tile([128, NFC, 128], bf16, tag="uw",
                                         name="uw")
                            nc.sync.dma_start(out=uw[:], in_=gu_blk[1, it])
                            for tch in range(THALF // 512):
                                t0 = th * THALF + tch * 512
                                gp = pmg.tile([128, 512], f32, tag="gp",
                                              name="gp")
                                up = pmu.tile([128, 512], f32, tag="up",
                                              name="up")
                                for fc in range(NFC):
                                    nc.tensor.matmul(
                                        gp[:], gw[:, fc, :],
                                        hnT[:, fc, t0:t0 + 512],
                                        start=(fc == 0),
                                        stop=(fc == NFC - 1))
                                    nc.tensor.matmul(
                                        up[:], uw[:, fc, :],
                                        hnT[:, fc, t0:t0 + 512],
                                        start=(fc == 0),
                                        stop=(fc == NFC - 1))
                                gs = pmd.tile([128, 512], bf16, tag="gs",
                                              name="gs")
                                nc.scalar.activation(gs[:], gp[:], AF.Silu)
                                nc.vector.tensor_mul(
                                    actT[:, it, tch * 512:(tch + 1) * 512],
                                    gs[:], up[:])
                    with tc.tile_pool(name="pmdn", bufs=1,
                                      space="PSUM") as pmdn:
                        for q8 in range(THALF // 128):
                            qt = th * (THALF // 128) + q8
                            dn_ps = pmdn.tile([128, H], f32, tag="dn",
                                              name="dn")
                            for it in range(NIT):
                                for nn in range(H // 512):
                                    nc.tensor.matmul(
                                        dn_ps[:, nn * 512:(nn + 1) * 512],
                                        actT[:, it,
                                             q8 * 128:(q8 + 1) * 128],
                                        dw_sb[:, it,
                                              nn * 512:(nn + 1) * 512],
                                        start=(it == 0),
                                        stop=(it == NIT - 1))
                            dn_bf = pmd.tile([128, H], bf16, tag="dn_bf",
                                             name="dn_bf")
                            nc.scalar.copy(dn_bf[:], dn_ps[:])
                            nc.sync.dma_start(out=g_mlp_in[qt], in_=dn_bf[:])
                nc.gpsimd.collective_compute(
                    "ReduceScatter", ALU.add, replica_groups=GRP,
                    ins=[g_mlp_in.opt()], outs=[g_mlp_out.opt()])
            with tc.tile_pool(name="pf", bufs=2) as pf, \
                 tc.tile_pool(name="pfs", bufs=1) as pfs:
                mlp_s = pfs.tile([128, NST, H], bf16, name="mlp_s")
                for st in range(NST):
                    nc.sync.dma_start(out=mlp_s[:, st, :], in_=g_mlp_out[st])
                for st in range(NST):
                    fin = pf.tile([128, H], f32, tag="fin", name="fin")
                    nc.vector.tensor_add(fin[:], h2_sb[:, st, :],
                                         mlp_s[:, st, :])
                    nc.sync.dma_start(out=out_strip[st], in_=fin[:])

        if MAXPH < 3:
            with tc.tile_pool(name="pex", bufs=2) as pex:
                for st in range(NST):
                    fin = pex.tile([128, H], f32, tag="finx", name="finx")
                    nc.vector.tensor_copy(fin[:], x_sb[:, st, :])
                    nc.sync.dma_start(out=out_strip[st], in_=fin[:])

        if os.environ.get("KERNEL_DEBUG"):
            def dump(name, gt, lead, width):
                dout = nc.dram_tensor(name, list(lead) + [128, width], bf16,
                                      kind="ExternalOutput").ap()
                import itertools
                with tc.tile_pool(name=f"dbg_{name}", bufs=2) as p:
                    for idx in itertools.product(*[range(d) for d in lead]):
                        t = p.tile([128, width], bf16, tag="t", name="t")
                        nc.sync.dma_start(out=t[:], in_=gt[idx])
                        nc.sync.dma_start(out=dout[idx], in_=t[:])
            dump("d_qcT", g_qcT_out, (NCORES, NST, NRC), 128)
            dump("d_kvT", g_kvT_out, (NCORES, NST, NKV + 1), 128)
            dump("d_ch", g_ch_out, (NCORES, NST), KVLR)
            if MAXPH >= 1:
                dump("d_at", g_at_out, (NST,), H)
                dump("d_atin", g_at_in, (NTT,), H)
            if MAXPH >= 2:
                dump("d_hnT", g_hnT_out, (NCORES, NST, NFC), 128)
            if MAXPH >= 3:
                dump("d_mlp", g_mlp_out, (NST,), H)
                dump("d_mlpin", g_mlp_in, (NTT,), H)
    nc.compile()
    return nc


def _host_prep(inputs):
    f32 = np.float32
    bf = bfloat16
    x = np.asarray(inputs["hidden_states"], f32)
    pos = np.asarray(inputs["positions"]).astype(f32)

    lnw_in = np.asarray(inputs["input_ln_w"], f32)
    q_a_w = np.asarray(inputs["q_a_w"], f32) * lnw_in[:, None]
    kv_a_w = np.asarray(inputs["kv_a_w"], f32) * lnw_in[:, None]
    q_b_w = (np.asarray(inputs["q_b_w"], f32)
             * np.asarray(inputs["q_a_ln_w"], f32)[:, None]) * SCALE
    kvln = np.asarray(inputs["kv_a_ln_w"], f32)
    w_uk = np.asarray(inputs["w_uk"], f32) * kvln[:, None, None]
    w_uv = np.asarray(inputs["w_uv"], f32) * kvln[:, None, None]
    o_w = np.asarray(inputs["o_w"], f32)
    pln = np.asarray(inputs["post_ln_w"], f32)
    gate_w = np.asarray(inputs["gate_w"], f32) * pln[:, None]
    up_w = np.asarray(inputs["up_w"], f32) * pln[:, None]
    down_w = np.asarray(inputs["down_w"], f32)

    inv_freq = 1.0 / (THETA ** (np.arange(0, DR, 2, dtype=f32) / DR))
    ang = pos[:, None] * inv_freq
    cos_t = np.cos(ang).astype(f32)
    sin_t = np.sin(ang).astype(f32)

    qb3 = q_b_w.reshape(QLR, NH, QH)
    wuk3 = w_uk.transpose(1, 2, 0)          # [NH, DN, KVLR]
    wuv3 = w_uv.transpose(1, 0, 2)          # [NH, KVLR, DV]
    ow3 = o_w.reshape(NH, DV, H)

    rep = {
        "qa_blk": np.ascontiguousarray(q_a_w.astype(bf).reshape(NFC, 128, QLR)),
        "kva_blk": np.ascontiguousarray(
            kv_a_w.astype(bf).reshape(NFC, 128, KVLR + DR)),
        "cosq": np.ascontiguousarray(cos_t.reshape(NTT, 128, DR // 2)),
        "sinq": np.ascontiguousarray(sin_t.reshape(NTT, 128, DR // 2)),
        "trimask": np.ascontiguousarray(
            np.triu(np.ones((128, 128), f32)).astype(bf)),
        "eye": np.eye(128, dtype=bf),
        "ones": np.ones((128, 1), bf),
    }

    per_core = []
    for c in range(NCORES):
        hs = [c * HPC + h for h in range(HPC)]
        i0 = c * IPC
        gpad = np.zeros((H, IPAD), f32)
        gpad[:, :IPC] = gate_w[:, i0:i0 + IPC]
        upad = np.zeros((H, IPAD), f32)
        upad[:, :IPC] = up_w[:, i0:i0 + IPC]
        dpad = np.zeros((IPAD, H), f32)
        dpad[:IPC] = down_w[i0:i0 + IPC]
        gu = np.stack([
            gpad.T.reshape(NIT, 128, NFC, 128).transpose(0, 3, 2, 1),
            upad.T.reshape(NIT, 128, NFC, 128).transpose(0, 3, 2, 1)])
        m = dict(rep)
        m["x_strip"] = np.ascontiguousarray(
            x[c * TS:(c + 1) * TS].reshape(NST, 128, H))
        m["qb_blk"] = np.ascontiguousarray(
            qb3[:, hs].reshape(QLR, HPC * QH).reshape(NRC, 128, HPC * QH)
            .astype(bf))
        m["wuk"] = np.ascontiguousarray(
            wuk3[hs].reshape(HPC, 128, NKV, 128).astype(bf))
        m["wuv"] = np.ascontiguousarray(
            wuv3[hs].reshape(HPC, NKV, 128, DV).transpose(0, 2, 1, 3)
            .astype(bf))
        m["ow_blk"] = np.ascontiguousarray(ow3[hs].astype(bf))
        m["gu_blk"] = np.ascontiguousarray(gu.astype(bf))
        m["dw_blk"] = np.ascontiguousarray(
            dpad.astype(bf).reshape(NIT, 128, H))
        m["cosk_s"] = np.ascontiguousarray(
            cos_t[c * TS:(c + 1) * TS].reshape(NST, 128, DR // 2))
        m["sink_s"] = np.ascontiguousarray(
            sin_t[c * TS:(c + 1) * TS].reshape(NST, 128, DR // 2))
        per_core.append(m)
    return per_core


def kernel(**inputs):
    from concourse import bass_utils

    if "nc" not in _CACHE:
        _CACHE["nc"] = _build_module()
    nc = _CACHE["nc"]

    import os
    in_maps = _host_prep(inputs)
    trace = bool(os.environ.get("BASS_KERNEL_TRACE"))
    res = bass_utils.run_bass_kernel_spmd(nc, in_maps,
                                          core_ids=list(range(NCORES)),
                                          trace=trace)
    if trace and res.exec_time_ns is not None:
        print(f"HW exec time: {res.exec_time_ns} ns")
        _CACHE["last_result"] = res
    out = np.zeros((T, H), np.float32)
    for c in range(NCORES):
        out[c * TS:(c + 1) * TS] = res.results[c]["out_strip"].reshape(TS, H)
    return out
